# revision 35
# baseline (speedup 1.0000x reference)
"""Trainium2 Bass kernel for a pre-LN transformer block (B=4, T=2048, D=1024, H=16).

Sharding: 8 cores = (batch b = core//2) x (half p = core%2). Each core handles
1024 query tokens of its batch: p=0 -> 512-token blocks {0,3}, p=1 -> {1,2}
(balanced causal work). K/V are recomputed per core from the full batch
sequence (no collectives). Per-core variation (token selection, causal masks)
is carried entirely in input data so one uniform SPMD program serves all cores.

Layout: feature-major ("transposed") activations [D, tokens] so every matmul
uses weights as stored (lhsT = W chunk), attention scores/AV need no on-chip
transposes, and softmax denominators come from a ones-column appended to V.

Precision: all matmuls bf16 with fp32 PSUM accumulation; LN statistics and
softmax reciprocal in fp32 (reciprocal_approx_fast, ~18 bits). Row broadcasts
(LN mu/rsig, softmax 1/den) are single-bf16 PE ones-matmuls.

Scheduling: attention pairs both heads of a head-pair into one [128,2,512]
PSUM score tile so one Exp activation covers both; scores are emitted one
kv-block ahead of the AV consume. The softmax denominator chain (den copy ->
approx reciprocal -> bf16 row -> raw copy) is deferred into the NEXT
head-pair's kv loop (chain at j==0, divide at j==5) so the in-order PE queue
never waits on the DVE row chain. reciprocal_approx_fast only works at
partition base 0, so denominators are first copied out of PSUM row 64
(cross-partition-base engine copies are fine; the custom DVE op is not).
"""

import sys

sys.path.insert(0, "/opt/trn_rl_repo")

import numpy as np
import ml_dtypes

import concourse.bass as bass
import concourse.mybir as mybir
import concourse.tile as tile
from concourse import bacc
from concourse.bass_utils import run_bass_kernel_spmd

BF16 = mybir.dt.bfloat16
F32 = mybir.dt.float32
AF = mybir.ActivationFunctionType

B, T, D, H, HD = 4, 2048, 1024, 16, 64
EPS = 1e-5
P = 128
DC = D // P            # 8 feature chunks
S = 2                  # q slots per core (512 tokens each)
TCKV = T // 512        # 4 kv token 512-chunks
NKV = [8, 16]          # kv 128-blocks per slot (uniform across cores)
FC = 4 * D // P        # 32 ffn hidden chunks
NMASK = 16
QBLOCKS = [[0, 3], [1, 2]]

_built = {}


def _masked(s, j):
    return (s == 0) or (j >= 8)


def build_nc():
    nc = bacc.Bacc("TRN2", target_bir_lowering=False, debug=False, num_devices=8)

    d = {}
    d["xkv"] = nc.dram_tensor("xkv", [DC, TCKV, P, 512], BF16, kind="ExternalInput").ap()
    d["xqb"] = nc.dram_tensor("xqb", [DC, S, P, 512], BF16, kind="ExternalInput").ap()
    d["xqf"] = nc.dram_tensor("xqf", [DC, S, P, 512], F32, kind="ExternalInput").ap()
    d["wq"] = nc.dram_tensor("wq", [DC, P, DC, P], BF16, kind="ExternalInput").ap()
    d["wk"] = nc.dram_tensor("wk", [DC, P, DC, P], BF16, kind="ExternalInput").ap()
    d["wo"] = nc.dram_tensor("wo", [DC, P, DC, P], BF16, kind="ExternalInput").ap()
    d["wv"] = nc.dram_tensor("wv", [DC, P, D], BF16, kind="ExternalInput").ap()
    d["w1"] = nc.dram_tensor("w1", [FC, P, DC, P], BF16, kind="ExternalInput").ap()
    d["w2"] = nc.dram_tensor("w2", [DC, P, FC, P], BF16, kind="ExternalInput").ap()
    d["masks"] = nc.dram_tensor("masks", [NMASK, P, 512], BF16, kind="ExternalInput").ap()
    d["biasg"] = nc.dram_tensor("biasg", [P, 6 * DC], F32, kind="ExternalInput").ap()
    d["b1c"] = nc.dram_tensor("b1c", [P, FC], F32, kind="ExternalInput").ap()
    d["onesc"] = nc.dram_tensor("onesc", [P, P], BF16, kind="ExternalInput").ap()
    d["ident"] = nc.dram_tensor("ident", [64, 64], BF16, kind="ExternalInput").ap()
    d["epsv"] = nc.dram_tensor("epsv", [1, 1], F32, kind="ExternalInput").ap()
    d["outT"] = nc.dram_tensor("outT", [DC, S, P, 512], F32, kind="ExternalOutput").ap()

    with tile.TileContext(nc) as tc:
        _emit(nc, tc, d)
    nc.compile()
    return nc


def _emit(nc, tc, d):
    from contextlib import ExitStack

    with ExitStack() as es:
        consts = es.enter_context(tc.tile_pool(name="consts", bufs=1))

        ones = consts.tile([P, P], BF16, tag="ones", name="ones")
        nc.sync.dma_start(ones[:], d["onesc"][:])
        ident = consts.tile([64, 64], BF16, tag="ident", name="ident")
        nc.sync.dma_start(ident[:], d["ident"][:])
        biasg = consts.tile([P, 6 * DC], F32, tag="biasg", name="biasg")
        nc.sync.dma_start(biasg[:], d["biasg"][:])
        b1t = consts.tile([P, FC], F32, tag="b1t", name="b1t")
        nc.sync.dma_start(b1t[:], d["b1c"][:])
        epst = consts.tile([1, 1], F32, tag="epst", name="epst")
        nc.sync.dma_start(epst[:], d["epsv"][:])

        bo_col = lambda dc: biasg[:, dc:dc + 1]
        g1_col = lambda dc: biasg[:, DC + dc:DC + dc + 1]
        bl1_col = lambda dc: biasg[:, 2 * DC + dc:2 * DC + dc + 1]
        g2_col = lambda dc: biasg[:, 3 * DC + dc:3 * DC + dc + 1]
        bl2_col = lambda dc: biasg[:, 4 * DC + dc:4 * DC + dc + 1]
        b2_col = lambda dc: biasg[:, 5 * DC + dc:5 * DC + dc + 1]

        def layer_norm(chunks, g_col, b_col, pools, pfx):
            """chunks: list of (src_fn() -> xb tiles per dc, h_tiles per dc).
            Software-pipelined: chunk i's broadcast/apply is emitted after
            chunk i+1's stats so PE never waits on the DVE row chain."""
            ps_st, ps_bc, p_rows, p_tmp, p_sq = pools

            def stats(ci, src_fn):
                xbt = src_fn()
                s1 = ps_st.tile([1, 512], F32, tag="st", name=f"{pfx}s1_{ci}")
                s2 = ps_st.tile([1, 512], F32, tag="st", name=f"{pfx}s2_{ci}")
                sqs = []
                for dc in range(DC):
                    sq = p_sq.tile([P, 512], BF16, tag="sq", name=f"{pfx}sq_{dc}_{ci}")
                    nc.scalar.square(sq[:], xbt[dc][:])
                    sqs.append(sq)
                for dc in range(DC):
                    nc.tensor.matmul(s1[:], ones[:, 0:1], xbt[dc][:],
                                     start=(dc == 0), stop=(dc == DC - 1))
                for dc in range(DC):
                    nc.tensor.matmul(s2[:], ones[:, 0:1], sqs[dc][:],
                                     start=(dc == 0), stop=(dc == DC - 1))
                mu = p_rows.tile([1, 512], F32, tag="rows", name=f"{pfx}mu_{ci}")
                nc.vector.tensor_scalar_mul(mu[:], s1[:], 1.0 / D)
                msq = p_rows.tile([1, 512], F32, tag="rows", name=f"{pfx}ms_{ci}")
                nc.vector.tensor_scalar_mul(msq[:], s2[:], 1.0 / D)
                var = p_rows.tile([1, 512], F32, tag="rows", name=f"{pfx}va_{ci}")
                nc.vector.tensor_mul(var[:], mu[:], mu[:])
                nc.vector.tensor_sub(var[:], msq[:], var[:])
                sd = p_rows.tile([1, 512], F32, tag="rows", name=f"{pfx}sd_{ci}")
                nc.scalar.activation(sd[:], var[:], AF.Sqrt, bias=epst[:])
                rsig = p_rows.tile([1, 512], F32, tag="rows", name=f"{pfx}rs_{ci}")
                nc.vector.reciprocal_approx_fast(rsig[:], sd[:])
                cmu = p_rows.tile([1, 512], F32, tag="rows", name=f"{pfx}cm_{ci}")
                nc.vector.tensor_mul(cmu[:], mu[:], rsig[:])
                ah = p_rows.tile([1, 512], BF16, tag="rowsb", name=f"{pfx}ah_{ci}")
                nc.vector.tensor_copy(ah[:], rsig[:])
                ch = p_rows.tile([1, 512], BF16, tag="rowsb", name=f"{pfx}ch_{ci}")
                nc.vector.tensor_copy(ch[:], cmu[:])
                return xbt, (ah, ch)

            def apply(ci, xbt, rows, h_tiles):
                ah, ch = rows
                bcA = ps_bc.tile([P, 512], F32, tag="bc", name=f"{pfx}bA_{ci}")
                nc.tensor.matmul(bcA[:], ones[0:1, :], ah[:], start=True, stop=True)
                bcC = ps_bc.tile([P, 512], F32, tag="bc", name=f"{pfx}bC_{ci}")
                nc.tensor.matmul(bcC[:], ones[0:1, :], ch[:], start=True, stop=True)
                for dc in range(DC):
                    tmp = p_tmp.tile([P, 512], F32, tag="lntmp", name=f"{pfx}lt_{dc}_{ci}")
                    nc.vector.tensor_mul(tmp[:], xbt[dc][:], bcA[:])
                    nc.vector.tensor_sub(tmp[:], tmp[:], bcC[:])
                    nc.scalar.activation(h_tiles[dc][:], tmp[:], AF.Identity,
                                         bias=b_col(dc), scale=g_col(dc))

            pend = None
            for ci, (src_fn, h_tiles) in enumerate(chunks):
                xbt, rows = stats(ci, src_fn)
                if pend is not None:
                    apply(*pend)
                pend = (ci, xbt, rows, h_tiles)
            apply(*pend)

        # ---------- persistent pool: tags reused across disjoint lifetimes ----
        # pa0..31: h (P1-2) then ff1 (P5); pa32..47: hq (P1-2)
        # pb0..31: kt (P2-3) then h2 (pb0..15) / xb2 (pb16..31) (P4-5)
        # pv0..15: v (P2-3) then x2 (P4-5);  pc0..15: qt;  pt0..15: att
        p_main = es.enter_context(tc.tile_pool(name="p_main", bufs=1))

        h_t = [[p_main.tile([P, 512], BF16, tag=f"pa{dc * TCKV + t}", name=f"h_{dc}_{t}")
                for t in range(TCKV)] for dc in range(DC)]
        hq_t = [[p_main.tile([P, 512], BF16, tag=f"pa{32 + dc * S + s}", name=f"hq_{dc}_{s}")
                 for s in range(S)] for dc in range(DC)]

        # ---------- phases 1+2: LN1 and projections (shared scope) ----------
        with tc.tile_pool(name="p_xsrc", bufs=16) as p_xsrc, \
             tc.tile_pool(name="ps_st", bufs=2, space="PSUM") as ps_st, \
             tc.tile_pool(name="ps_bc", bufs=2, space="PSUM") as ps_bc, \
             tc.tile_pool(name="p_rows", bufs=4) as p_rows, \
             tc.tile_pool(name="p_tmp", bufs=4) as p_tmp, \
             tc.tile_pool(name="p_sq", bufs=8) as p_sq, \
             tc.tile_pool(name="p_wsl", bufs=2) as p_wsl, \
             tc.tile_pool(name="p_wvs", bufs=9) as p_wvs, \
             tc.tile_pool(name="ps_mm", bufs=4, space="PSUM") as ps_mm:

            def mk_src_kv(tcx):
                def f():
                    out = []
                    for dc in range(DC):
                        xt = p_xsrc.tile([P, 512], BF16, tag="xsrc", name=f"xkv_{dc}_{tcx}")
                        nc.sync.dma_start(xt[:], d["xkv"][dc, tcx])
                        out.append(xt)
                    return out
                return f

            def mk_src_q(s):
                def f():
                    out = []
                    for dc in range(DC):
                        xt = p_xsrc.tile([P, 512], BF16, tag="xsrc", name=f"xqb_{dc}_{s}")
                        nc.sync.dma_start(xt[:], d["xqb"][dc, s])
                        out.append(xt)
                    return out
                return f

            pools = (ps_st, ps_bc, p_rows, p_tmp, p_sq)
            chunks = [(mk_src_kv(t), [h_t[dc][t] for dc in range(DC)]) for t in range(TCKV)]
            chunks += [(mk_src_q(s), [hq_t[dc][s] for dc in range(DC)]) for s in range(S)]
            layer_norm(chunks, g1_col, bl1_col, pools, "a")

            # ------- projections KT, V, QT (overlap LN1) -------
            kt_t = [[p_main.tile([P, 512], BF16, tag=f"pb{dc * TCKV + t}", name=f"kt_{dc}_{t}")
                     for t in range(TCKV)] for dc in range(DC)]
            v_t = [p_main.tile([P, H * 65], BF16, tag=f"pv{j}", name=f"v_{j}")
                   for j in range(T // P)]
            qt_t = [[p_main.tile([P, 512], BF16, tag=f"pc{dc * S + s}", name=f"qt_{dc}_{s}")
                     for s in range(S)] for dc in range(DC)]
            for dc in range(DC):  # K^T
                wsl = p_wsl.tile([P, DC, P], BF16, tag="wk", name=f"wks_{dc}")
                nc.sync.dma_start(wsl[:], d["wk"][dc])
                for t in range(TCKV):
                    pt = ps_mm.tile([P, 512], F32, tag="mm", name=f"pk_{dc}_{t}")
                    for di in range(DC):
                        nc.tensor.matmul(pt[:], wsl[:, di], h_t[di][t][:],
                                         start=(di == 0), stop=(di == DC - 1))
                    nc.scalar.copy(kt_t[dc][t][:], pt[:])
            for j in range(T // P):  # ones columns of V
                nc.scalar.activation(
                    v_t[j].rearrange("p (h c) -> p h c", c=65)[:, :, 64:65],
                    ones[:, 0:H].unsqueeze(2), AF.Copy)
            for doc in range(2):  # V token-major
                wvs = [p_wvs.tile([P, 512], BF16, tag="wv", name=f"wvs_{di}_{doc}")
                       for di in range(DC)]
                for di in range(DC):
                    nc.sync.dma_start(wvs[di][:], d["wv"][di, :, doc * 512:(doc + 1) * 512])
                for j in range(T // P):
                    t5, jo = j // 4, (j % 4) * P
                    pt = ps_mm.tile([P, 512], F32, tag="mm", name=f"pv_{j}_{doc}")
                    for di in range(DC):
                        nc.tensor.matmul(pt[:], h_t[di][t5][:, jo:jo + P], wvs[di][:],
                                         start=(di == 0), stop=(di == DC - 1))
                    dst = v_t[j].rearrange("p (h c) -> p h c", c=65)[:, doc * 8:(doc + 1) * 8, 0:64]
                    nc.scalar.copy(dst, pt.rearrange("p (h c) -> p h c", c=64))
            for dc in range(DC):  # Q^T
                wsl = p_wsl.tile([P, DC, P], BF16, tag="wq", name=f"wqs_{dc}")
                nc.sync.dma_start(wsl[:], d["wq"][dc])
                for s in range(S):
                    pt = ps_mm.tile([P, 512], F32, tag="mm", name=f"pq_{dc}_{s}")
                    for di in range(DC):
                        nc.tensor.matmul(pt[:], wsl[:, di], hq_t[di][s][:],
                                         start=(di == 0), stop=(di == DC - 1))
                    nc.scalar.copy(qt_t[dc][s][:], pt[:])

        # ---------- phase 3: attention ----------
        att_t = [[p_main.tile([P, 512], BF16, tag=f"pt{dc * S + s}", name=f"at_{dc}_{s}")
                  for s in range(S)] for dc in range(DC)]
        # slot-0 residual x+attn_out, bf16, produced by O(s0) units interleaved
        # into the s=1 attention stream as PE filler
        x2b0 = [p_main.tile([P, 512], BF16, tag=f"pa{32 + dc}", name=f"x2b0_{dc}")
                for dc in range(DC)]

        with tc.tile_pool(name="ps_s", bufs=2, space="PSUM") as ps_s, \
             tc.tile_pool(name="ps_av", bufs=3, space="PSUM") as ps_av, \
             tc.tile_pool(name="ps_rb", bufs=1, space="PSUM") as ps_rb, \
             tc.tile_pool(name="p_es", bufs=6) as p_es, \
             tc.tile_pool(name="p_raw", bufs=5) as p_raw, \
             tc.tile_pool(name="p_rrow", bufs=4) as p_rrow, \
             tc.tile_pool(name="p_ow", bufs=2) as p_ow, \
             tc.tile_pool(name="p_oxq", bufs=2) as p_oxq, \
             tc.tile_pool(name="p_msk", bufs=1) as p_msk:
            maskt = []
            for mi in range(NMASK):
                mt = p_msk.tile([P, 512], BF16, tag=f"mask{mi}", name=f"mask{mi}")
                nc.sync.dma_start(mt[:], d["masks"][mi])
                maskt.append(mt)

            def sc_emit(s, hp, j):
                sp = ps_s.tile([P, 2, 512], F32, tag="sc", name=f"sc_{s}_{hp}_{j}")
                for hh in range(2):
                    lo = hh * 64
                    nc.tensor.matmul(
                        sp[:, hh], kt_t[hp][j // 4][lo:lo + 64, (j % 4) * P:(j % 4) * P + P],
                        qt_t[hp][s][lo:lo + 64, :], start=True, stop=True)
                return sp

            def flush_chain(st):
                s, hp, avp = st["s"], st["hp"], st["avp"]
                for hh in range(2):
                    head = 2 * hp + hh
                    den = p_rrow.tile([1, 512], F32, tag="den", name=f"dn_{s}_{head}", bufs=2)
                    nc.vector.tensor_copy(den[:], avp[hh][64:65, :])
                    rec = p_rrow.tile([1, 512], F32, tag="rr", name=f"rc_{s}_{head}", bufs=2)
                    nc.vector.reciprocal_approx_fast(rec[:], den[:])
                    rh = p_rrow.tile([1, 512], BF16, tag="rrb", name=f"rh_{s}_{head}", bufs=4)
                    nc.vector.tensor_copy(rh[:], rec[:])
                    raw = p_raw.tile([64, 512], BF16, tag="raw", name=f"rw_{s}_{head}")
                    if s == 0:
                        # s0 is DVE-period-bound; scalar has slack there
                        nc.scalar.copy(raw[:], avp[hh][0:64, :])
                    else:
                        nc.vector.tensor_copy(raw[:], avp[hh][0:64, :])
                    st["raws"].append(raw)
                    st["rows"].append(rh)

            def flush_div(st):
                s, hp = st["s"], st["hp"]
                for hh in range(2):
                    head = 2 * hp + hh
                    rh = st["rows"][hh]
                    raw = st["raws"][hh]
                    rb = ps_rb.tile([64, 512], F32, tag="rb", name=f"rb_{s}_{head}")
                    nc.tensor.matmul(rb[:], ones[0:1, 0:64], rh[:],
                                     start=True, stop=True)
                    if hh == 0:
                        nc.vector.tensor_mul(att_t[hp][s][0:64, :], raw[:], rb[:])
                    else:
                        sc1 = p_raw.tile([64, 512], BF16, tag="scm", name=f"sm_{s}_{head}", bufs=2)
                        nc.vector.tensor_mul(sc1[:], raw[:], rb[:])
                        nc.gpsimd.tensor_copy(att_t[hp][s][64:128, :], sc1[:])

            prev = None
            for s in range(S):
                for hp in range(DC):
                    avp = [ps_av.tile([65, 512], F32, tag="av", name=f"av_{s}_{hp}_{hh}")
                           for hh in range(2)]
                    st = {"s": s, "hp": hp, "avp": avp, "raws": [], "rows": []}
                    sp_cur = sc_emit(s, hp, 0)
                    for j in range(NKV[s]):
                        es_ = p_es.tile([P, 2, 512], BF16, tag="es", name=f"es_{s}_{hp}_{j}")
                        nc.scalar.activation(es_[:], sp_cur[:], AF.Exp, scale=HD ** -0.5)
                        if _masked(s, j):
                            for hh in range(2):
                                nc.vector.tensor_mul(es_[:, hh], es_[:, hh], maskt[j][:])
                        if j == 0 and prev is not None:
                            flush_chain(prev)
                        if j == 5 and prev is not None:
                            flush_div(prev)
                        if s == 1 and j == 2:
                            owsl = p_ow.tile([P, DC, P], BF16, tag="ow", name=f"ow_{hp}")
                            nc.sync.dma_start(owsl[:], d["wo"][hp])
                            oxq = p_oxq.tile([P, 512], F32, tag="oxq", name=f"oxq_{hp}")
                            nc.sync.dma_start(oxq[:], d["xqf"][hp, 0])
                        if s == 1 and j == 10:
                            pt0 = ps_rb.tile([P, 512], F32, tag="rb", name=f"po0_{hp}")
                            for di in range(DC):
                                nc.tensor.matmul(pt0[:], owsl[:, di], att_t[di][0][:],
                                                 start=(di == 0), stop=(di == DC - 1))
                            nc.vector.scalar_tensor_tensor(
                                x2b0[hp][:], pt0[:], bo_col(hp), oxq[:],
                                mybir.AluOpType.add, mybir.AluOpType.add)
                        if j + 1 < NKV[s]:
                            sp_cur = sc_emit(s, hp, j + 1)
                        for hh in range(2):
                            nc.tensor.matmul(
                                avp[hh][:],
                                v_t[j].rearrange("p (h c) -> p h c", c=65)[:, 2 * hp + hh],
                                es_[:, hh], start=(j == 0), stop=(j == NKV[s] - 1))
                    prev = st
            flush_chain(prev)
            flush_div(prev)

        # ---------- phase 4: O-projection + residual + LN2 ----------
        x2_t = [[p_main.tile([P, 512], F32, tag=f"pv{dc * S + s}", name=f"x2_{dc}_{s}")
                 for s in range(S)] for dc in range(DC)]
        h2_t = [[p_main.tile([P, 512], BF16, tag=f"pb{dc * S + s}", name=f"h2_{dc}_{s}")
                 for s in range(S)] for dc in range(DC)]

        with tc.tile_pool(name="p_wsl2", bufs=2) as p_wsl2, \
             tc.tile_pool(name="p_xqf", bufs=3) as p_xqf, \
             tc.tile_pool(name="p_otmp", bufs=4) as p_otmp, \
             tc.tile_pool(name="ps_mm2", bufs=4, space="PSUM") as ps_mm2, \
             tc.tile_pool(name="ps_st2", bufs=2, space="PSUM") as ps_st2, \
             tc.tile_pool(name="ps_bc2", bufs=2, space="PSUM") as ps_bc2, \
             tc.tile_pool(name="p_rows2", bufs=3) as p_rows2, \
             tc.tile_pool(name="p_tmp2", bufs=4) as p_tmp2, \
             tc.tile_pool(name="p_sq2", bufs=8) as p_sq2:
            xb2_t = [[p_main.tile([P, 512], BF16, tag=f"pb{16 + dc * S + s}", name=f"xb2_{dc}_{s}")
                      for s in range(S)] for dc in range(DC)]
            for dc in range(DC):
                wsl = p_wsl2.tile([P, DC, P], BF16, tag="wo", name=f"wos_{dc}")
                nc.sync.dma_start(wsl[:], d["wo"][dc])
                for s in (1,):
                    pt = ps_mm2.tile([P, 512], F32, tag="mm2", name=f"po_{dc}_{s}")
                    for di in range(DC):
                        nc.tensor.matmul(pt[:], wsl[:, di], att_t[di][s][:],
                                         start=(di == 0), stop=(di == DC - 1))
                    ot = p_otmp.tile([P, 512], F32, tag="ot", name=f"o_{dc}_{s}")
                    nc.scalar.activation(ot[:], pt[:], AF.Identity, bias=bo_col(dc))
                    xqf = p_xqf.tile([P, 512], F32, tag="xqf", name=f"xqf_{dc}_{s}")
                    nc.sync.dma_start(xqf[:], d["xqf"][dc, s])
                    nc.vector.tensor_add(x2_t[dc][s][:], xqf[:], ot[:])
                    nc.scalar.copy(xb2_t[dc][s][:], x2_t[dc][s][:])
            pools2 = (ps_st2, ps_bc2, p_rows2, p_tmp2, p_sq2)
            chunks2 = [(lambda: [x2b0[dc] for dc in range(DC)],
                        [h2_t[dc][0] for dc in range(DC)]),
                       (lambda: [xb2_t[dc][1] for dc in range(DC)],
                        [h2_t[dc][1] for dc in range(DC)])]
            layer_norm(chunks2, g2_col, bl2_col, pools2, "c")

        # ---------- phase 5: FFN + residual + output ----------
        with tc.tile_pool(name="p_w1s", bufs=3) as p_w1s, \
             tc.tile_pool(name="p_w2s", bufs=2) as p_w2s, \
             tc.tile_pool(name="p_fout", bufs=4) as p_fout, \
             tc.tile_pool(name="p_out", bufs=4) as p_out, \
             tc.tile_pool(name="ps_mm3", bufs=6, space="PSUM") as ps_mm3:
            ff1_t = [p_main.tile([P, 512], BF16, tag=f"pa{fc}", name=f"ff1_{fc}")
                     for fc in range(FC)]
            for s in range(S):
                for fc in range(FC):
                    w1s = p_w1s.tile([P, DC, P], BF16, tag="w1s", name=f"w1s_{s}_{fc}")
                    nc.sync.dma_start(w1s[:], d["w1"][fc])
                    pt = ps_mm3.tile([P, 512], F32, tag="mm3", name=f"pf_{fc}_{s}")
                    for di in range(DC):
                        nc.tensor.matmul(pt[:], w1s[:, di], h2_t[di][s][:],
                                         start=(di == 0), stop=(di == DC - 1))
                    nc.scalar.activation(ff1_t[fc][:], pt[:], AF.Relu,
                                         bias=b1t[:, fc:fc + 1])
                for dc in range(DC):
                    w2s = p_w2s.tile([P, FC, P], BF16, tag="w2s", name=f"w2s_{s}_{dc}")
                    nc.sync.dma_start(w2s[:], d["w2"][dc])
                    pt = ps_mm3.tile([P, 512], F32, tag="mm3", name=f"pg_{dc}_{s}")
                    for fc in range(FC):
                        nc.tensor.matmul(pt[:], w2s[:, fc], ff1_t[fc][:],
                                         start=(fc == 0), stop=(fc == FC - 1))
                    f2 = p_fout.tile([P, 512], F32, tag="f2", name=f"f2_{dc}_{s}")
                    nc.scalar.activation(f2[:], pt[:], AF.Relu, bias=b2_col(dc))
                    ou = p_out.tile([P, 512], F32, tag="ou", name=f"ou_{dc}_{s}")
                    xres = x2b0[dc] if s == 0 else x2_t[dc][s]
                    nc.vector.tensor_add(ou[:], xres[:], f2[:])
                    nc.sync.dma_start(d["outT"][dc, s], ou[:])


# ============================ host side ============================

def _slab(w, rows_chunks, cols_chunks):
    r, c = w.shape
    return np.ascontiguousarray(
        w.reshape(rows_chunks, r // rows_chunks, cols_chunks, c // cols_chunks)
        .transpose(2, 1, 0, 3)).astype(ml_dtypes.bfloat16)


def _prep_core(inputs, core):
    b, p = core // 2, core % 2
    bf16 = ml_dtypes.bfloat16
    x = np.asarray(inputs["x"], np.float32)[b]
    xT = np.ascontiguousarray(x.T)
    qb = QBLOCKS[p]
    qidx = np.concatenate([np.arange(q_ * 512, q_ * 512 + 512) for q_ in qb])
    xqT = np.ascontiguousarray(xT[:, qidx])

    m = {}
    m["xkv"] = np.ascontiguousarray(
        xT.reshape(DC, P, TCKV, 512).transpose(0, 2, 1, 3)).astype(bf16)
    xq4 = np.ascontiguousarray(xqT.reshape(DC, P, S, 512).transpose(0, 2, 1, 3))
    m["xqb"] = xq4.astype(bf16)
    m["xqf"] = xq4.astype(np.float32)
    m["wq"] = _slab(np.asarray(inputs["Wq"], np.float32), DC, DC)
    m["wk"] = _slab(np.asarray(inputs["Wk"], np.float32), DC, DC)
    m["wo"] = _slab(np.asarray(inputs["Wo"], np.float32), DC, DC)
    m["wv"] = np.ascontiguousarray(
        np.asarray(inputs["Wv"], np.float32).reshape(DC, P, D)).astype(bf16)
    m["w1"] = _slab(np.asarray(inputs["W1"], np.float32), DC, FC)
    m["w2"] = _slab(np.asarray(inputs["W2"], np.float32), FC, DC)

    masks = np.zeros((NMASK, P, 512), np.float32)
    for s in range(S):
        qstart = qb[s] * 512
        for j in (range(8) if s == 0 else range(8, 16)):
            kv = j * P + np.arange(P)[:, None]
            qg = qstart + np.arange(512)[None, :]
            masks[j] = (kv <= qg).astype(np.float32)
    m["masks"] = masks.astype(bf16)

    biasg = np.zeros((P, 6 * DC), np.float32)
    for i, key in enumerate(["bo", "ln1_g", "ln1_b", "ln2_g", "ln2_b", "b2"]):
        biasg[:, i * DC:(i + 1) * DC] = np.asarray(inputs[key], np.float32).reshape(DC, P).T
    m["biasg"] = np.ascontiguousarray(biasg)
    m["b1c"] = np.ascontiguousarray(
        np.asarray(inputs["b1"], np.float32).reshape(FC, P).T)
    m["onesc"] = np.ones((P, P), bf16)
    m["ident"] = np.eye(64, dtype=np.float32).astype(bf16)
    m["epsv"] = np.full((1, 1), EPS, np.float32)
    return m


def kernel(**inputs):
    if "nc" not in _built:
        _built["nc"] = build_nc()
    nc = _built["nc"]
    in_maps = [_prep_core(inputs, c) for c in range(8)]
    res = run_bass_kernel_spmd(nc, in_maps, core_ids=list(range(8)))
    out = np.zeros((B, T, D), np.float32)
    for c in range(8):
        b, p = c // 2, c % 2
        o = np.asarray(res.results[c]["outT"])
        for s in range(S):
            qb = QBLOCKS[p][s]
            blk = o[:, s].reshape(D, 512)
            out[b, qb * 512:(qb + 1) * 512, :] = blk.T
    return out.astype(np.float32)



# revision 36
# speedup vs baseline: 1.0122x; 1.0122x over previous
"""Trainium2 Bass kernel for a pre-LN transformer block (B=4, T=2048, D=1024, H=16).

Sharding: 8 cores = (batch b = core//2) x (half p = core%2). Each core handles
1024 query tokens of its batch: p=0 -> 512-token blocks {0,3}, p=1 -> {1,2}
(balanced causal work). K/V are recomputed per core from the full batch
sequence (no collectives). Per-core variation (token selection, causal masks)
is carried entirely in input data so one uniform SPMD program serves all cores.

Layout: feature-major ("transposed") activations [D, tokens] so every matmul
uses weights as stored (lhsT = W chunk), attention scores/AV need no on-chip
transposes, and softmax denominators come from a ones-column appended to V.

Precision: all matmuls bf16 with fp32 PSUM accumulation; LN statistics and
softmax reciprocal in fp32 (reciprocal_approx_fast, ~18 bits). Row broadcasts
(LN mu/rsig, softmax 1/den) are single-bf16 PE ones-matmuls.

Scheduling: attention pairs both heads of a head-pair into one [128,2,512]
PSUM score tile so one Exp activation covers both; scores are emitted one
kv-block ahead of the AV consume. The softmax denominator chain (den copy ->
approx reciprocal -> bf16 row -> raw copy) is deferred into the NEXT
head-pair's kv loop (chain at j==0, divide at j==5) so the in-order PE queue
never waits on the DVE row chain. reciprocal_approx_fast only works at
partition base 0, so denominators are first copied out of PSUM row 64
(cross-partition-base engine copies are fine; the custom DVE op is not).
"""

import sys

sys.path.insert(0, "/opt/trn_rl_repo")

import numpy as np
import ml_dtypes

import concourse.bass as bass
import concourse.mybir as mybir
import concourse.tile as tile
from concourse import bacc
from concourse.bass_utils import run_bass_kernel_spmd

BF16 = mybir.dt.bfloat16
F32 = mybir.dt.float32
AF = mybir.ActivationFunctionType

B, T, D, H, HD = 4, 2048, 1024, 16, 64
EPS = 1e-5
P = 128
DC = D // P            # 8 feature chunks
S = 2                  # q slots per core (512 tokens each)
TCKV = T // 512        # 4 kv token 512-chunks
NKV = [8, 16]          # kv 128-blocks per slot (uniform across cores)
FC = 4 * D // P        # 32 ffn hidden chunks
NMASK = 16
QBLOCKS = [[0, 3], [1, 2]]

_built = {}


def _masked(s, j):
    return (s == 0) or (j >= 8)


def build_nc():
    nc = bacc.Bacc("TRN2", target_bir_lowering=False, debug=False, num_devices=8)

    d = {}
    d["xkv"] = nc.dram_tensor("xkv", [DC, TCKV, P, 512], BF16, kind="ExternalInput").ap()
    d["xqb"] = nc.dram_tensor("xqb", [DC, S, P, 512], BF16, kind="ExternalInput").ap()
    d["xqf"] = nc.dram_tensor("xqf", [DC, S, P, 512], F32, kind="ExternalInput").ap()
    d["wq"] = nc.dram_tensor("wq", [DC, P, DC, P], BF16, kind="ExternalInput").ap()
    d["wk"] = nc.dram_tensor("wk", [DC, P, DC, P], BF16, kind="ExternalInput").ap()
    d["wo"] = nc.dram_tensor("wo", [DC, P, DC, P], BF16, kind="ExternalInput").ap()
    d["wv"] = nc.dram_tensor("wv", [DC, P, D], BF16, kind="ExternalInput").ap()
    d["w1"] = nc.dram_tensor("w1", [FC, P, DC, P], BF16, kind="ExternalInput").ap()
    d["w2"] = nc.dram_tensor("w2", [DC, P, FC, P], BF16, kind="ExternalInput").ap()
    d["masks"] = nc.dram_tensor("masks", [NMASK, P, 512], BF16, kind="ExternalInput").ap()
    d["biasg"] = nc.dram_tensor("biasg", [P, 6 * DC], F32, kind="ExternalInput").ap()
    d["b1c"] = nc.dram_tensor("b1c", [P, FC], F32, kind="ExternalInput").ap()
    d["onesc"] = nc.dram_tensor("onesc", [P, P], BF16, kind="ExternalInput").ap()
    d["ident"] = nc.dram_tensor("ident", [64, 64], BF16, kind="ExternalInput").ap()
    d["epsv"] = nc.dram_tensor("epsv", [1, 1], F32, kind="ExternalInput").ap()
    d["outT"] = nc.dram_tensor("outT", [DC, S, P, 512], F32, kind="ExternalOutput").ap()

    with tile.TileContext(nc) as tc:
        _emit(nc, tc, d)
    nc.compile()
    return nc


def _emit(nc, tc, d):
    from contextlib import ExitStack

    with ExitStack() as es:
        consts = es.enter_context(tc.tile_pool(name="consts", bufs=1))

        ones = consts.tile([P, P], BF16, tag="ones", name="ones")
        nc.sync.dma_start(ones[:], d["onesc"][:])
        ident = consts.tile([64, 64], BF16, tag="ident", name="ident")
        nc.sync.dma_start(ident[:], d["ident"][:])
        biasg = consts.tile([P, 6 * DC], F32, tag="biasg", name="biasg")
        nc.sync.dma_start(biasg[:], d["biasg"][:])
        b1t = consts.tile([P, FC], F32, tag="b1t", name="b1t")
        nc.sync.dma_start(b1t[:], d["b1c"][:])
        epst = consts.tile([1, 1], F32, tag="epst", name="epst")
        nc.sync.dma_start(epst[:], d["epsv"][:])

        bo_col = lambda dc: biasg[:, dc:dc + 1]
        g1_col = lambda dc: biasg[:, DC + dc:DC + dc + 1]
        bl1_col = lambda dc: biasg[:, 2 * DC + dc:2 * DC + dc + 1]
        g2_col = lambda dc: biasg[:, 3 * DC + dc:3 * DC + dc + 1]
        bl2_col = lambda dc: biasg[:, 4 * DC + dc:4 * DC + dc + 1]
        b2_col = lambda dc: biasg[:, 5 * DC + dc:5 * DC + dc + 1]

        def layer_norm(chunks, g_col, b_col, pools, pfx):
            """chunks: list of (src_fn() -> xb tiles per dc, h_tiles per dc).
            Software-pipelined: chunk i's broadcast/apply is emitted after
            chunk i+1's stats so PE never waits on the DVE row chain."""
            ps_st, ps_bc, p_rows, p_tmp, p_sq = pools

            def stats(ci, src_fn):
                xbt = src_fn()
                s1 = ps_st.tile([1, 512], F32, tag="st", name=f"{pfx}s1_{ci}")
                s2 = ps_st.tile([1, 512], F32, tag="st", name=f"{pfx}s2_{ci}")
                sqs = []
                for dc in range(DC):
                    sq = p_sq.tile([P, 512], BF16, tag="sq", name=f"{pfx}sq_{dc}_{ci}")
                    nc.scalar.square(sq[:], xbt[dc][:])
                    sqs.append(sq)
                for dc in range(DC):
                    nc.tensor.matmul(s1[:], ones[:, 0:1], xbt[dc][:],
                                     start=(dc == 0), stop=(dc == DC - 1))
                for dc in range(DC):
                    nc.tensor.matmul(s2[:], ones[:, 0:1], sqs[dc][:],
                                     start=(dc == 0), stop=(dc == DC - 1))
                mu = p_rows.tile([1, 512], F32, tag="rows", name=f"{pfx}mu_{ci}")
                nc.vector.tensor_scalar_mul(mu[:], s1[:], 1.0 / D)
                msq = p_rows.tile([1, 512], F32, tag="rows", name=f"{pfx}ms_{ci}")
                nc.vector.tensor_scalar_mul(msq[:], s2[:], 1.0 / D)
                var = p_rows.tile([1, 512], F32, tag="rows", name=f"{pfx}va_{ci}")
                nc.vector.tensor_mul(var[:], mu[:], mu[:])
                nc.vector.tensor_sub(var[:], msq[:], var[:])
                sd = p_rows.tile([1, 512], F32, tag="rows", name=f"{pfx}sd_{ci}")
                nc.scalar.activation(sd[:], var[:], AF.Sqrt, bias=epst[:])
                rsig = p_rows.tile([1, 512], F32, tag="rows", name=f"{pfx}rs_{ci}")
                nc.vector.reciprocal_approx_fast(rsig[:], sd[:])
                cmu = p_rows.tile([1, 512], F32, tag="rows", name=f"{pfx}cm_{ci}")
                nc.vector.tensor_mul(cmu[:], mu[:], rsig[:])
                ah = p_rows.tile([1, 512], BF16, tag="rowsb", name=f"{pfx}ah_{ci}")
                nc.vector.tensor_copy(ah[:], rsig[:])
                ch = p_rows.tile([1, 512], BF16, tag="rowsb", name=f"{pfx}ch_{ci}")
                nc.vector.tensor_copy(ch[:], cmu[:])
                return xbt, (ah, ch)

            def apply(ci, xbt, rows, h_tiles):
                ah, ch = rows
                bcA = ps_bc.tile([P, 512], F32, tag="bc", name=f"{pfx}bA_{ci}")
                nc.tensor.matmul(bcA[:], ones[0:1, :], ah[:], start=True, stop=True)
                bcC = ps_bc.tile([P, 512], F32, tag="bc", name=f"{pfx}bC_{ci}")
                nc.tensor.matmul(bcC[:], ones[0:1, :], ch[:], start=True, stop=True)
                for dc in range(DC):
                    tmp = p_tmp.tile([P, 512], F32, tag="lntmp", name=f"{pfx}lt_{dc}_{ci}")
                    nc.vector.tensor_mul(tmp[:], xbt[dc][:], bcA[:])
                    nc.vector.tensor_sub(tmp[:], tmp[:], bcC[:])
                    nc.scalar.activation(h_tiles[dc][:], tmp[:], AF.Identity,
                                         bias=b_col(dc), scale=g_col(dc))

            pend = None
            for ci, (src_fn, h_tiles) in enumerate(chunks):
                xbt, rows = stats(ci, src_fn)
                if pend is not None:
                    apply(*pend)
                pend = (ci, xbt, rows, h_tiles)
            apply(*pend)

        # ---------- persistent pool: tags reused across disjoint lifetimes ----
        # pa0..31: h (P1-2) then ff1 (P5); pa32..47: hq (P1-2)
        # pb0..31: kt (P2-3) then h2 (pb0..15) / xb2 (pb16..31) (P4-5)
        # pv0..15: v (P2-3) then x2 (P4-5);  pc0..15: qt;  pt0..15: att
        p_main = es.enter_context(tc.tile_pool(name="p_main", bufs=1))

        h_t = [[p_main.tile([P, 512], BF16, tag=f"pa{dc * TCKV + t}", name=f"h_{dc}_{t}")
                for t in range(TCKV)] for dc in range(DC)]
        hq_t = [[p_main.tile([P, 512], BF16, tag=f"pa{32 + dc * S + s}", name=f"hq_{dc}_{s}")
                 for s in range(S)] for dc in range(DC)]

        # ---------- phases 1+2: LN1 and projections (shared scope) ----------
        with tc.tile_pool(name="p_xsrc", bufs=16) as p_xsrc, \
             tc.tile_pool(name="ps_st", bufs=2, space="PSUM") as ps_st, \
             tc.tile_pool(name="ps_bc", bufs=2, space="PSUM") as ps_bc, \
             tc.tile_pool(name="p_rows", bufs=4) as p_rows, \
             tc.tile_pool(name="p_tmp", bufs=4) as p_tmp, \
             tc.tile_pool(name="p_sq", bufs=8) as p_sq, \
             tc.tile_pool(name="p_wsl", bufs=2) as p_wsl, \
             tc.tile_pool(name="p_wvs", bufs=9) as p_wvs, \
             tc.tile_pool(name="ps_mm", bufs=4, space="PSUM") as ps_mm:

            def mk_src_kv(tcx):
                def f():
                    out = []
                    for dc in range(DC):
                        xt = p_xsrc.tile([P, 512], BF16, tag="xsrc", name=f"xkv_{dc}_{tcx}")
                        nc.sync.dma_start(xt[:], d["xkv"][dc, tcx])
                        out.append(xt)
                    return out
                return f

            def mk_src_q(s):
                def f():
                    out = []
                    for dc in range(DC):
                        xt = p_xsrc.tile([P, 512], BF16, tag="xsrc", name=f"xqb_{dc}_{s}")
                        nc.sync.dma_start(xt[:], d["xqb"][dc, s])
                        out.append(xt)
                    return out
                return f

            pools = (ps_st, ps_bc, p_rows, p_tmp, p_sq)
            chunks = [(mk_src_kv(t), [h_t[dc][t] for dc in range(DC)]) for t in range(TCKV)]
            chunks += [(mk_src_q(s), [hq_t[dc][s] for dc in range(DC)]) for s in range(S)]
            layer_norm(chunks, g1_col, bl1_col, pools, "a")

            # ------- projections KT, V, QT (overlap LN1) -------
            kt_t = [[p_main.tile([P, 512], BF16, tag=f"pb{dc * TCKV + t}", name=f"kt_{dc}_{t}")
                     for t in range(TCKV)] for dc in range(DC)]
            v_t = [p_main.tile([P, H * 65], BF16, tag=f"pv{j}", name=f"v_{j}")
                   for j in range(T // P)]
            qt_t = [[p_main.tile([P, 512], BF16, tag=f"pc{dc * S + s}", name=f"qt_{dc}_{s}")
                     for s in range(S)] for dc in range(DC)]
            for dc in range(DC):  # K^T
                wsl = p_wsl.tile([P, DC, P], BF16, tag="wk", name=f"wks_{dc}")
                nc.sync.dma_start(wsl[:], d["wk"][dc])
                for t in range(TCKV):
                    pt = ps_mm.tile([P, 512], F32, tag="mm", name=f"pk_{dc}_{t}")
                    for di in range(DC):
                        nc.tensor.matmul(pt[:], wsl[:, di], h_t[di][t][:],
                                         start=(di == 0), stop=(di == DC - 1))
                    nc.scalar.copy(kt_t[dc][t][:], pt[:])
            for j in range(T // P):  # ones columns of V
                nc.scalar.activation(
                    v_t[j].rearrange("p (h c) -> p h c", c=65)[:, :, 64:65],
                    ones[:, 0:H].unsqueeze(2), AF.Copy)
            for doc in range(2):  # V token-major
                wvs = [p_wvs.tile([P, 512], BF16, tag="wv", name=f"wvs_{di}_{doc}")
                       for di in range(DC)]
                for di in range(DC):
                    nc.sync.dma_start(wvs[di][:], d["wv"][di, :, doc * 512:(doc + 1) * 512])
                for j in range(T // P):
                    t5, jo = j // 4, (j % 4) * P
                    pt = ps_mm.tile([P, 512], F32, tag="mm", name=f"pv_{j}_{doc}")
                    for di in range(DC):
                        nc.tensor.matmul(pt[:], h_t[di][t5][:, jo:jo + P], wvs[di][:],
                                         start=(di == 0), stop=(di == DC - 1))
                    dst = v_t[j].rearrange("p (h c) -> p h c", c=65)[:, doc * 8:(doc + 1) * 8, 0:64]
                    nc.scalar.copy(dst, pt.rearrange("p (h c) -> p h c", c=64))
            for dc in range(DC):  # Q^T
                wsl = p_wsl.tile([P, DC, P], BF16, tag="wq", name=f"wqs_{dc}")
                nc.sync.dma_start(wsl[:], d["wq"][dc])
                for s in range(S):
                    pt = ps_mm.tile([P, 512], F32, tag="mm", name=f"pq_{dc}_{s}")
                    for di in range(DC):
                        nc.tensor.matmul(pt[:], wsl[:, di], hq_t[di][s][:],
                                         start=(di == 0), stop=(di == DC - 1))
                    nc.scalar.copy(qt_t[dc][s][:], pt[:])

        # ---------- phase 3: attention ----------
        att_t = [[p_main.tile([P, 512], BF16, tag=f"pt{dc * S + s}", name=f"at_{dc}_{s}")
                  for s in range(S)] for dc in range(DC)]
        # slot-0 residual x+attn_out, bf16, produced by O(s0) units interleaved
        # into the s=1 attention stream as PE filler
        x2b0 = [p_main.tile([P, 512], BF16, tag=f"pa{32 + dc}", name=f"x2b0_{dc}")
                for dc in range(DC)]

        with tc.tile_pool(name="ps_s", bufs=2, space="PSUM") as ps_s, \
             tc.tile_pool(name="ps_av", bufs=3, space="PSUM") as ps_av, \
             tc.tile_pool(name="ps_rb", bufs=1, space="PSUM") as ps_rb, \
             tc.tile_pool(name="p_es", bufs=6) as p_es, \
             tc.tile_pool(name="p_raw", bufs=5) as p_raw, \
             tc.tile_pool(name="p_rrow", bufs=4) as p_rrow, \
             tc.tile_pool(name="p_ow", bufs=2) as p_ow, \
             tc.tile_pool(name="p_oxq", bufs=2) as p_oxq, \
             tc.tile_pool(name="p_msk", bufs=1) as p_msk:
            maskt = []
            for mi in range(NMASK):
                mt = p_msk.tile([P, 512], BF16, tag=f"mask{mi}", name=f"mask{mi}")
                nc.sync.dma_start(mt[:], d["masks"][mi])
                maskt.append(mt)

            def sc_emit(s, hp, j):
                sp = ps_s.tile([P, 2, 512], F32, tag="sc", name=f"sc_{s}_{hp}_{j}")
                for hh in range(2):
                    lo = hh * 64
                    nc.tensor.matmul(
                        sp[:, hh], kt_t[hp][j // 4][lo:lo + 64, (j % 4) * P:(j % 4) * P + P],
                        qt_t[hp][s][lo:lo + 64, :], start=True, stop=True)
                return sp

            def flush_chain(st):
                s, hp, avp = st["s"], st["hp"], st["avp"]
                for hh in range(2):
                    head = 2 * hp + hh
                    den = p_rrow.tile([1, 512], F32, tag="den", name=f"dn_{s}_{head}", bufs=2)
                    nc.vector.tensor_copy(den[:], avp[hh][64:65, :])
                    rec = p_rrow.tile([1, 512], F32, tag="rr", name=f"rc_{s}_{head}", bufs=2)
                    nc.vector.reciprocal_approx_fast(rec[:], den[:])
                    rh = p_rrow.tile([1, 512], BF16, tag="rrb", name=f"rh_{s}_{head}", bufs=4)
                    nc.vector.tensor_copy(rh[:], rec[:])
                    raw = p_raw.tile([64, 512], BF16, tag="raw", name=f"rw_{s}_{head}")
                    nc.vector.tensor_copy(raw[:], avp[hh][0:64, :])
                    st["raws"].append(raw)
                    st["rows"].append(rh)

            def flush_div(st):
                s, hp = st["s"], st["hp"]
                for hh in range(2):
                    head = 2 * hp + hh
                    rh = st["rows"][hh]
                    raw = st["raws"][hh]
                    rb = ps_rb.tile([64, 512], F32, tag="rb", name=f"rb_{s}_{head}")
                    nc.tensor.matmul(rb[:], ones[0:1, 0:64], rh[:],
                                     start=True, stop=True)
                    if hh == 0:
                        nc.vector.tensor_mul(att_t[hp][s][0:64, :], raw[:], rb[:])
                    else:
                        sc1 = p_raw.tile([64, 512], BF16, tag="scm", name=f"sm_{s}_{head}", bufs=2)
                        nc.vector.tensor_mul(sc1[:], raw[:], rb[:])
                        nc.vector.tensor_copy(att_t[hp][s][64:128, :], sc1[:])

            prev = None
            for s in range(S):
                for hp in range(DC):
                    avp = [ps_av.tile([65, 512], F32, tag="av", name=f"av_{s}_{hp}_{hh}")
                           for hh in range(2)]
                    st = {"s": s, "hp": hp, "avp": avp, "raws": [], "rows": []}
                    sp_cur = sc_emit(s, hp, 0)
                    for j in range(NKV[s]):
                        es_ = p_es.tile([P, 2, 512], BF16, tag="es", name=f"es_{s}_{hp}_{j}")
                        nc.scalar.activation(es_[:], sp_cur[:], AF.Exp, scale=HD ** -0.5)
                        if _masked(s, j):
                            for hh in range(2):
                                nc.vector.tensor_mul(es_[:, hh], es_[:, hh], maskt[j][:])
                        if j == 0 and prev is not None:
                            flush_chain(prev)
                        if j == 5 and prev is not None:
                            flush_div(prev)
                        if s == 1 and j == 2:
                            owsl = p_ow.tile([P, DC, P], BF16, tag="ow", name=f"ow_{hp}")
                            nc.sync.dma_start(owsl[:], d["wo"][hp])
                            oxq = p_oxq.tile([P, 512], F32, tag="oxq", name=f"oxq_{hp}")
                            nc.sync.dma_start(oxq[:], d["xqf"][hp, 0])
                        if s == 1 and j == 10:
                            pt0 = ps_rb.tile([P, 512], F32, tag="rb", name=f"po0_{hp}")
                            for di in range(DC):
                                nc.tensor.matmul(pt0[:], owsl[:, di], att_t[di][0][:],
                                                 start=(di == 0), stop=(di == DC - 1))
                            nc.vector.scalar_tensor_tensor(
                                x2b0[hp][:], pt0[:], bo_col(hp), oxq[:],
                                mybir.AluOpType.add, mybir.AluOpType.add)
                        if j + 1 < NKV[s]:
                            sp_cur = sc_emit(s, hp, j + 1)
                        for hh in range(2):
                            nc.tensor.matmul(
                                avp[hh][:],
                                v_t[j].rearrange("p (h c) -> p h c", c=65)[:, 2 * hp + hh],
                                es_[:, hh], start=(j == 0), stop=(j == NKV[s] - 1))
                    prev = st
            flush_chain(prev)
            flush_div(prev)

        # ---------- phase 4: O-projection + residual + LN2 ----------
        x2_t = [[p_main.tile([P, 512], F32, tag=f"pv{dc * S + s}", name=f"x2_{dc}_{s}")
                 for s in range(S)] for dc in range(DC)]
        h2_t = [[p_main.tile([P, 512], BF16, tag=f"pb{dc * S + s}", name=f"h2_{dc}_{s}")
                 for s in range(S)] for dc in range(DC)]

        with tc.tile_pool(name="p_wsl2", bufs=2) as p_wsl2, \
             tc.tile_pool(name="p_xqf", bufs=3) as p_xqf, \
             tc.tile_pool(name="p_otmp", bufs=4) as p_otmp, \
             tc.tile_pool(name="ps_mm2", bufs=4, space="PSUM") as ps_mm2, \
             tc.tile_pool(name="ps_st2", bufs=2, space="PSUM") as ps_st2, \
             tc.tile_pool(name="ps_bc2", bufs=2, space="PSUM") as ps_bc2, \
             tc.tile_pool(name="p_rows2", bufs=3) as p_rows2, \
             tc.tile_pool(name="p_tmp2", bufs=4) as p_tmp2, \
             tc.tile_pool(name="p_sq2", bufs=8) as p_sq2:
            xb2_t = [[p_main.tile([P, 512], BF16, tag=f"pb{16 + dc * S + s}", name=f"xb2_{dc}_{s}")
                      for s in range(S)] for dc in range(DC)]
            for dc in range(DC):
                wsl = p_wsl2.tile([P, DC, P], BF16, tag="wo", name=f"wos_{dc}")
                nc.sync.dma_start(wsl[:], d["wo"][dc])
                for s in (1,):
                    pt = ps_mm2.tile([P, 512], F32, tag="mm2", name=f"po_{dc}_{s}")
                    for di in range(DC):
                        nc.tensor.matmul(pt[:], wsl[:, di], att_t[di][s][:],
                                         start=(di == 0), stop=(di == DC - 1))
                    ot = p_otmp.tile([P, 512], F32, tag="ot", name=f"o_{dc}_{s}")
                    nc.scalar.activation(ot[:], pt[:], AF.Identity, bias=bo_col(dc))
                    xqf = p_xqf.tile([P, 512], F32, tag="xqf", name=f"xqf_{dc}_{s}")
                    nc.sync.dma_start(xqf[:], d["xqf"][dc, s])
                    nc.vector.tensor_add(x2_t[dc][s][:], xqf[:], ot[:])
                    nc.scalar.copy(xb2_t[dc][s][:], x2_t[dc][s][:])
            pools2 = (ps_st2, ps_bc2, p_rows2, p_tmp2, p_sq2)
            chunks2 = [(lambda: [x2b0[dc] for dc in range(DC)],
                        [h2_t[dc][0] for dc in range(DC)]),
                       (lambda: [xb2_t[dc][1] for dc in range(DC)],
                        [h2_t[dc][1] for dc in range(DC)])]
            layer_norm(chunks2, g2_col, bl2_col, pools2, "c")

        # ---------- phase 5: FFN + residual + output ----------
        with tc.tile_pool(name="p_w1s", bufs=3) as p_w1s, \
             tc.tile_pool(name="p_w2s", bufs=2) as p_w2s, \
             tc.tile_pool(name="p_fout", bufs=4) as p_fout, \
             tc.tile_pool(name="p_out", bufs=4) as p_out, \
             tc.tile_pool(name="ps_mm3", bufs=6, space="PSUM") as ps_mm3:
            ff1_t = [p_main.tile([P, 512], BF16, tag=f"pa{fc}", name=f"ff1_{fc}")
                     for fc in range(FC)]
            for s in range(S):
                for fc in range(FC):
                    w1s = p_w1s.tile([P, DC, P], BF16, tag="w1s", name=f"w1s_{s}_{fc}")
                    nc.sync.dma_start(w1s[:], d["w1"][fc])
                    pt = ps_mm3.tile([P, 512], F32, tag="mm3", name=f"pf_{fc}_{s}")
                    for di in range(DC):
                        nc.tensor.matmul(pt[:], w1s[:, di], h2_t[di][s][:],
                                         start=(di == 0), stop=(di == DC - 1))
                    nc.scalar.activation(ff1_t[fc][:], pt[:], AF.Relu,
                                         bias=b1t[:, fc:fc + 1])
                for dc in range(DC):
                    w2s = p_w2s.tile([P, FC, P], BF16, tag="w2s", name=f"w2s_{s}_{dc}")
                    nc.sync.dma_start(w2s[:], d["w2"][dc])
                    pt = ps_mm3.tile([P, 512], F32, tag="mm3", name=f"pg_{dc}_{s}")
                    for fc in range(FC):
                        nc.tensor.matmul(pt[:], w2s[:, fc], ff1_t[fc][:],
                                         start=(fc == 0), stop=(fc == FC - 1))
                    f2 = p_fout.tile([P, 512], F32, tag="f2", name=f"f2_{dc}_{s}")
                    nc.scalar.activation(f2[:], pt[:], AF.Relu, bias=b2_col(dc))
                    ou = p_out.tile([P, 512], F32, tag="ou", name=f"ou_{dc}_{s}")
                    xres = x2b0[dc] if s == 0 else x2_t[dc][s]
                    nc.vector.tensor_add(ou[:], xres[:], f2[:])
                    nc.sync.dma_start(d["outT"][dc, s], ou[:])


# ============================ host side ============================

def _slab(w, rows_chunks, cols_chunks):
    r, c = w.shape
    return np.ascontiguousarray(
        w.reshape(rows_chunks, r // rows_chunks, cols_chunks, c // cols_chunks)
        .transpose(2, 1, 0, 3)).astype(ml_dtypes.bfloat16)


def _prep_core(inputs, core):
    b, p = core // 2, core % 2
    bf16 = ml_dtypes.bfloat16
    x = np.asarray(inputs["x"], np.float32)[b]
    xT = np.ascontiguousarray(x.T)
    qb = QBLOCKS[p]
    qidx = np.concatenate([np.arange(q_ * 512, q_ * 512 + 512) for q_ in qb])
    xqT = np.ascontiguousarray(xT[:, qidx])

    m = {}
    m["xkv"] = np.ascontiguousarray(
        xT.reshape(DC, P, TCKV, 512).transpose(0, 2, 1, 3)).astype(bf16)
    xq4 = np.ascontiguousarray(xqT.reshape(DC, P, S, 512).transpose(0, 2, 1, 3))
    m["xqb"] = xq4.astype(bf16)
    m["xqf"] = xq4.astype(np.float32)
    m["wq"] = _slab(np.asarray(inputs["Wq"], np.float32), DC, DC)
    m["wk"] = _slab(np.asarray(inputs["Wk"], np.float32), DC, DC)
    m["wo"] = _slab(np.asarray(inputs["Wo"], np.float32), DC, DC)
    m["wv"] = np.ascontiguousarray(
        np.asarray(inputs["Wv"], np.float32).reshape(DC, P, D)).astype(bf16)
    m["w1"] = _slab(np.asarray(inputs["W1"], np.float32), DC, FC)
    m["w2"] = _slab(np.asarray(inputs["W2"], np.float32), FC, DC)

    masks = np.zeros((NMASK, P, 512), np.float32)
    for s in range(S):
        qstart = qb[s] * 512
        for j in (range(8) if s == 0 else range(8, 16)):
            kv = j * P + np.arange(P)[:, None]
            qg = qstart + np.arange(512)[None, :]
            masks[j] = (kv <= qg).astype(np.float32)
    m["masks"] = masks.astype(bf16)

    biasg = np.zeros((P, 6 * DC), np.float32)
    for i, key in enumerate(["bo", "ln1_g", "ln1_b", "ln2_g", "ln2_b", "b2"]):
        biasg[:, i * DC:(i + 1) * DC] = np.asarray(inputs[key], np.float32).reshape(DC, P).T
    m["biasg"] = np.ascontiguousarray(biasg)
    m["b1c"] = np.ascontiguousarray(
        np.asarray(inputs["b1"], np.float32).reshape(FC, P).T)
    m["onesc"] = np.ones((P, P), bf16)
    m["ident"] = np.eye(64, dtype=np.float32).astype(bf16)
    m["epsv"] = np.full((1, 1), EPS, np.float32)
    return m


def kernel(**inputs):
    if "nc" not in _built:
        _built["nc"] = build_nc()
    nc = _built["nc"]
    in_maps = [_prep_core(inputs, c) for c in range(8)]
    res = run_bass_kernel_spmd(nc, in_maps, core_ids=list(range(8)))
    out = np.zeros((B, T, D), np.float32)
    for c in range(8):
        b, p = c // 2, c % 2
        o = np.asarray(res.results[c]["outT"])
        for s in range(S):
            qb = QBLOCKS[p][s]
            blk = o[:, s].reshape(D, 512)
            out[b, qb * 512:(qb + 1) * 512, :] = blk.T
    return out.astype(np.float32)

